# revision 12
# baseline (speedup 1.0000x reference)
"""Trainium2 Bass kernel for a 12-head attention block (B=2, N=2048, C=768).

Sharding: the 24 (batch, head) pairs are split across 8 NeuronCores —
4 cores per batch element, 3 heads per core (data + head/tensor parallel).
Each core computes qkv projections for its heads, the full attention for
its heads (the N x N score matrix is private to a core), and a *partial*
output projection over its heads' channels.  The host sums the 4 partial
projections per batch element (the tensor-parallel all-reduce) and adds
the bias.

Device algorithm (activations/weights bf16 — the PE streams its moving
operand at 1 column/cycle for 2-byte dtypes vs 2 cycles for fp32/fp32r —
with fp32 PSUM accumulation everywhere):

  xT [768, 2048] (x transposed on host)
  B:  qk^T  = W_qk^T.T @ xT  -> per-head tile [q^T(64 rows); k^T(64)] x 2048
      (attention scale 1/8 and b_q, b_k folded into W/bias on host)
  B2: v     = xT.T @ W_v^T   -> [2048, 3*65] with a column of ones per head
  C:  S^T[key, q] = k^T.T @ q^T        (per 128-key tile, 512-q chunk)
      P^T = exp(S^T)                   (ScalarE, no max subtraction:
                                        logits are in [-3, 3] by construction)
      ctx_u^T[d|den, q] += [v | 1].T @ P^T   (fused denominator row)
  D:  ctx^T = ctx_u^T[0:64] * (1/den)  (den broadcast across partitions via a
      ones-row matmul at base partition 64, then reciprocal_approx_fast)
  E:  y[n, :] += ctx^T.T @ W_p^T      (partial projection, summed on host)

Scheduling notes (why the emission order looks the way it does): the PE's
HAM clock gate re-throttles 2.4->1.2 GHz after ~3.4us of idle, so the
normalize/projection work for a tile is deferred and emitted mid-way
through the NEXT attention block, where the exp pipeline has buffered
work; a warm-up matmul spin bridges the input-DMA window; the exp table
set is preloaded with a dummy activation; S^T matmul pairs are emitted
back-to-back into opposite PE row groups so they overlap.
"""

import numpy as np
import ml_dtypes

import concourse.bacc as bacc
import concourse.tile as tile
import concourse.mybir as mybir
from concourse.bass_utils import run_bass_kernel_spmd

# Problem shape (hardcoded; harness contract)
B, N, C = 2, 2048, 768
H, HD = 12, 64
NCORES = 8
CORES_PER_B = NCORES // B      # 4
HPC = H // CORES_PER_B         # 3 heads per core
P = 128
NT = N // P                    # 16 key/n tiles
KT = C // P                    # 6 c_in tiles
CH = 512                       # q chunk (max fp32 moving free dim)
QCH = N // CH                  # 4 chunks
VW = 3 * 65                    # v width: 3 heads x (64 + fused ones column)

f32 = mybir.dt.float32
f32r = mybir.dt.float32r
bf16 = mybir.dt.bfloat16
EXP = mybir.ActivationFunctionType.Exp

def _emit(tc, nc, xT, w_qk, w_v, b_qk, w_p, vones, onesrow, y, dbg=None):
    from contextlib import ExitStack

    with ExitStack() as ctx:
        consts = ctx.enter_context(tc.tile_pool(name="consts", bufs=1))
        qk_pool = ctx.enter_context(tc.tile_pool(name="qk", bufs=HPC))
        qk2_pool = ctx.enter_context(tc.tile_pool(name="qk2", bufs=HPC))
        v_pool = ctx.enter_context(tc.tile_pool(name="v", bufs=NT))
        ctx_pool = ctx.enter_context(tc.tile_pool(name="ctxp", bufs=HPC))
        y_pool = ctx.enter_context(tc.tile_pool(name="y", bufs=3))
        ps_a = ctx.enter_context(tc.tile_pool(name="ps_a", bufs=2, space="PSUM"))
        ps_s = ctx.enter_context(tc.tile_pool(name="ps_s", bufs=2, space="PSUM"))
        ps_c = ctx.enter_context(tc.tile_pool(name="ps_c", bufs=2, space="PSUM"))

        # ---- memset-only tiles first (no DMA deps): the PE warm-up spin and
        # the exp table preload key off these, so both start right at the
        # preamble boundary instead of waiting for any input DMA.
        warm_sb = consts.tile([P, 2 * P], bf16)
        nc.vector.memset(warm_sb[:], 0.0)
        # persistent denominator-row tiles, zeroed once: the broadcast matmul
        # reads all 64 partitions (64:128) so the unused rows must be 0, not
        # SBUF garbage (NaN x 0 = NaN)
        denr_tiles = [
            consts.tile([P, CH], bf16, name=f"denr{_}") for _ in range(2)
        ]
        for t in denr_tiles:
            nc.vector.memset(t[:], 0.0)

        # preload the exp spline table set (~2.7us): FIRST instruction on the
        # Activation engine, before that engine's input DMAs
        actwarm = consts.tile([P, P], bf16)
        nc.scalar.activation(actwarm[:], warm_sb[:, 0:P], EXP)

        # ---- inputs, split across the two HWDGE queues (SP + Activation).
        # The per-core HBM wire (~360 GB/s) is shared by both queues, so the
        # ~4.3MB of input takes ~12us on the wire no matter what; the ordering
        # below puts each tensor on the wire in need-time order.  [128,1024]
        # half-chunks keep the 2KB-per-partition-line full DMA rate (quarter
        # chunks with 1KB lines measured at half rate).
        wqk_sb = consts.tile([P, KT, 2 * HD * HPC], bf16)
        nc.sync.dma_start(wqk_sb[:], w_qk.rearrange("(t p) m -> p t m", p=P))

        x_pool = ctx.enter_context(tc.tile_pool(name="x", bufs=KT))
        x_sb = [x_pool.tile([P, N], bf16, tag="x", name=f"x{_}") for _ in range(KT)]
        half = N // 2
        for kt in range(KT):
            nc.sync.dma_start(x_sb[kt][:, 0:half], xT[kt * P : (kt + 1) * P, 0:half])
        vones_sb = consts.tile([P, VW], bf16)
        nc.sync.dma_start(vones_sb[:], vones[:])

        # scalar (Activation) queue: tiny bias first, v weights (JIT v
        # projections consume them mid-first-block), x second halves, then
        # the late-needed projection constants.  b_qk is host-transposed to
        # [128, 3] so the transfer is one clean descriptor per partition (a
        # strided 4-byte gather here poisons the queue for ~2.4us).
        bqk_sb = consts.tile([P, HPC], f32)
        nc.scalar.dma_start(bqk_sb[:], b_qk[:])
        wv_sb = consts.tile([P, KT, VW], bf16)
        nc.scalar.dma_start(wv_sb[:], w_v.rearrange("(t p) m -> p t m", p=P))
        for kt in range(KT):
            nc.scalar.dma_start(
                x_sb[kt][:, half:N], xT[kt * P : (kt + 1) * P, half:N]
            )
        # projection weights, host-packed [128, 2C]: cols 0:C = heads 0+1
        # stacked; cols C:2C = head 2 zero-padded to 128 contraction rows so
        # every E matmul runs in (128,128) PE mode
        wp_sb = consts.tile([P, 2 * C], bf16)
        nc.scalar.dma_start(wp_sb[:], w_p[:])
        # [128,128] broadcast stationary: row 64 = [ones(64) | zeros(64)],
        # everything else zero.  Shaped so the den-broadcast matmul runs in
        # the same (64,128) PE tile mode as the S^T pairs / projection units
        # (row-tiling mode switches drain the PE array).
        onesrow_sb = consts.tile([P, P], bf16)
        nc.scalar.dma_start(onesrow_sb[:], onesrow[:])

        # persistent activations
        qk_sb = [qk_pool.tile([P, N], bf16, tag="qk", name=f"qk{_}") for _ in range(HPC)]
        qk2_sb = [qk2_pool.tile([P, N], bf16, tag="qk2", name=f"qk2_{_}") for _ in range(HPC)]
        v_sb = [v_pool.tile([P, VW], bf16, tag="v", name=f"v{_}") for _ in range(NT)]
        ctx01_sb = ctx_pool.tile([P, N], bf16, tag="ctx01", name="ctx01")
        ctx2_sb = ctx_pool.tile([P, N], bf16, tag="ctx2", name="ctx2")
        stage_pool = ctx.enter_context(tc.tile_pool(name="stage", bufs=2))

        nc.vector.memset(ctx2_sb[HD:P, :], 0.0)

        # PE warm-up: the HAM clock gate needs ~3.4us of sustained matmul
        # activity to lift the PE from 1.2 to 2.4 GHz, and re-throttles after
        # a ~3.4us idle window.  Spin dummy matmuls on the memset tile (no DMA
        # dep) so the spin starts at the preamble boundary; sized to end right
        # as the first x chunks land off the parallel DMA queues.
        wps = ps_a.tile([P, CH], f32, tag="ps_a", name="warm_ps")
        for _ in range(22):
            nc.tensor.matmul(
                wps[:, 0 : 2 * P], warm_sb[:, 0:P], warm_sb[:], start=True, stop=True
            )

        def emit_qk_group(t, cc):
            # qk^T head tile t, q-chunk cc: [q^T(64); k^T(64)] x CH
            sl = slice(cc * CH, (cc + 1) * CH)
            ps = ps_a.tile([P, CH], f32, tag="ps_a", name="ps_qk")
            for kt in range(KT):
                nc.tensor.matmul(
                    ps[:],
                    wqk_sb[:, kt, t * P : (t + 1) * P],
                    x_sb[kt][:, sl],
                    start=(kt == 0),
                    stop=(kt == KT - 1),
                )
            nc.vector.tensor_scalar_add(qk_sb[t][:, sl], ps[:], bqk_sb[:, t : t + 1])
            # swapped copy per chunk (k^T to partitions 0:64, q^T to 64:128);
            # on the GPSIMD SWDGE queue: SBUF->SBUF, so it costs no HBM wire
            # time and never queues behind the x/weight input stream
            nc.gpsimd.dma_start(qk2_sb[t][0:HD, sl], qk_sb[t][HD:P, sl])
            nc.gpsimd.dma_start(qk2_sb[t][HD:P, sl], qk_sb[t][0:HD, sl])

        def emit_v(nt):
            # v natural layout [key, 3*65] (+ ones columns)
            ps = ps_a.tile([P, CH], f32, tag="ps_a", name="ps_v")
            for kt in range(KT):
                nc.tensor.matmul(
                    ps[:, 0:VW],
                    x_sb[kt][:, nt * P : (nt + 1) * P],
                    wv_sb[:, kt, :],
                    start=(kt == 0),
                    stop=(kt == KT - 1),
                )
            nc.vector.tensor_add(v_sb[nt][:], ps[:, 0:VW], vones_sb[:])

        bc_pool = ctx.enter_context(tc.tile_pool(name="bc", bufs=2))
        p_pool = ctx.enter_context(tc.tile_pool(name="p", bufs=8))

        # ---- C/D/E: attention, normalize (deferred one step), projection.
        # S^T pairs go to opposite PE row groups (partitions 0:64 / 64:128)
        # and run concurrently; exp processes both halves in one ACTIVATE.
        # The normalize (D) for tile (c,h) is emitted only after the next
        # C-block so its reciprocal never stalls the PE (a >3.4us PE gap
        # re-throttles the HAM clock gate to 1.2 GHz).
        # deferred-ctx slot: (cps, h, kp, pt) of the S^T pair whose P@V
        # matmuls have not been emitted yet.  Deferring the ctx of pair j
        # until after pair j+1's S^T matmuls gives EXP(j) a full iteration
        # of slack, so the first ctx matmul never stalls on the exp sem.
        pending_ctx = [None]

        def emit_ctx(cps, h, kp, pt):
            kt0, kt1 = 2 * kp, 2 * kp + 1
            nc.tensor.matmul(
                cps[:],
                v_sb[kt0][:, h * 65 : (h + 1) * 65],
                pt[:, 0:CH],
                start=(kp == 0),
                stop=False,
            )
            nc.tensor.matmul(
                cps[:],
                v_sb[kt1][:, h * 65 : (h + 1) * 65],
                pt[:, CH : 2 * CH],
                start=False,
                stop=(kp == NT // 2 - 1),
            )

        def emit_C(c, h, first=False, flush=None, qk_feed=None):
            cps = ps_c.tile([65, CH], f32, tag="ps_c", name="cps")
            for kp in range(NT // 2):
                kt0, kt1 = 2 * kp, 2 * kp + 1
                if qk_feed is not None and kp in (1, 3, 5):
                    # first block: the remaining q-chunk projections are
                    # emitted just ahead of the S^T pairs that consume their
                    # key columns, so the block starts after chunk 0 lands
                    qk_feed(kp // 2 + 1)
                if first and kp >= 1:
                    # just-in-time v projection (v0/v1 are emitted before the
                    # C loop while the q/k swap DMA is still in flight)
                    emit_v(kt0)
                    emit_v(kt1)
                sps = ps_s.tile([P, 2 * CH], f32, tag="ps_s", name="sps")
                nc.tensor.matmul(
                    sps[:, 0:CH],
                    qk2_sb[h][0:HD, kt0 * P : (kt0 + 1) * P],
                    qk_sb[h][0:HD, c * CH : (c + 1) * CH],
                )
                nc.tensor.matmul(
                    sps[:, CH : 2 * CH],
                    qk_sb[h][HD:P, kt1 * P : (kt1 + 1) * P],
                    qk2_sb[h][HD:P, c * CH : (c + 1) * CH],
                )
                pt = p_pool.tile([P, 2 * CH], bf16, tag="p", name="pt")
                nc.scalar.activation(pt[:], sps[:], EXP)
                if pending_ctx[0] is not None:
                    emit_ctx(*pending_ctx[0])
                pending_ctx[0] = (cps, h, kp, pt)
                if kp % 2 == 1 and flush is not None:
                    # emit the previous tile's normalize / projection units
                    # here, in the (128,128) PE mode window after the ctx
                    # matmuls (mode switches drain the array).  Not at kp 0:
                    # ACT has no buffered work at a block start, so PE detours
                    # there starve the exp stream.  The last head's blocks
                    # drain the E backlog at double rate so the post-EXP tail
                    # only carries the final chunk's units.
                    flush(2 if h == HPC - 1 else 1)
            return cps

        dcount = [0]

        def emit_D(c, h, cps):
            denr = denr_tiles[dcount[0] % 2]
            dcount[0] += 1
            nc.vector.tensor_copy(denr[64:65, :], cps[64:65, :])
            bps = ps_a.tile([P, CH], f32, tag="ps_a", name="bps")
            # contraction padded to 128 rows (onesrow/denr rows other than 64
            # are zero) so the broadcast runs in (128,128) PE mode like the
            # ctx / projection matmuls — no row-tiling mode switch
            nc.tensor.matmul(
                bps[:], onesrow_sb[:], denr[:],
                start=True, stop=True,
            )
            bc = bc_pool.tile([HD, CH], f32, tag="bc", name="bc")
            # reciprocal straight from PSUM (custom DVE ops accept PSUM
            # sources) — drops the staging copy from the D critical chain
            nc.vector.reciprocal_approx_fast(bc[:], bps[0:HD, :])
            sl = slice(c * CH, (c + 1) * CH)
            if h == 0:
                nc.vector.tensor_mul(ctx01_sb[0:HD, sl], cps[0:HD, :], bc[:])
            elif h == 1:
                # stage + DMA into partitions 64:128 of the stacked tile
                # (DVE lanes are partition-locked; only DMA crosses halves)
                stg = stage_pool.tile([HD, CH], bf16, tag="stg", name="stg")
                nc.vector.tensor_mul(stg[:], cps[0:HD, :], bc[:])
                nc.gpsimd.dma_start(ctx01_sb[HD:P, sl], stg[:])
            else:
                nc.vector.tensor_mul(ctx2_sb[0:HD, sl], cps[0:HD, :], bc[:])

        def emit_E_unit(nt):
            # partial projection for one n-tile: heads 0+1 contract 128 deep
            # via the stacked ctx01 tile; head 2 is zero-padded to 128 rows
            psA = ps_a.tile([P, CH], f32, tag="ps_a", name="psA")
            psB = ps_a.tile([P, CH], f32, tag="ps_a", name="psB")
            nsl = slice(nt * P, (nt + 1) * P)
            nc.tensor.matmul(psA[:], ctx01_sb[:, nsl], wp_sb[:, 0:CH], start=True, stop=False)
            nc.tensor.matmul(psA[:], ctx2_sb[:, nsl], wp_sb[:, C : C + CH], start=False, stop=True)
            nc.tensor.matmul(psB[:, 0 : C - CH], ctx01_sb[:, nsl], wp_sb[:, CH:C], start=True, stop=False)
            nc.tensor.matmul(psB[:, 0 : C - CH], ctx2_sb[:, nsl], wp_sb[:, C + CH : 2 * C], start=False, stop=True)
            ysb = y_pool.tile([P, C], bf16, tag="y", name="ysb")
            nc.vector.tensor_copy(ysb[:, 0:CH], psA[:])
            # psB evacuation on DVE too: keeps the (bottleneck) ACT engine's
            # queue free for the exp stream
            nc.vector.tensor_copy(ysb[:, CH:C], psB[:, 0 : C - CH])
            nc.sync.dma_start(y[nt * P : (nt + 1) * P, :], ysb[:])

        pending = [None]  # (c, h, cps) awaiting D
        pending_E = []  # (c, nt) projection units ready to emit

        def flush_pending(budget=1):
            if pending[0] is not None:
                pc, ph, pcps = pending[0]
                emit_D(pc, ph, pcps)
                if ph == HPC - 1:
                    pending_E.extend(
                        pc * (CH // P) + i for i in range(CH // P)
                    )
                pending[0] = None
                budget -= 1
            while budget > 0 and pending_E:
                emit_E_unit(pending_E.pop(0))
                budget -= 1

        # Head-outer schedule: all 4 q-chunks of head h, then head h+1.
        # The next head's qk projection is emitted one chunk-group at a time
        # underneath the current (ACT-bound) attention blocks.
        emit_qk_group(0, 0)
        emit_v(0)
        emit_v(1)
        for h in range(HPC):
            for c in range(QCH):
                cps = emit_C(
                    c, h, first=(h == 0 and c == 0), flush=flush_pending,
                    qk_feed=(lambda cc: emit_qk_group(0, cc)) if (h == 0 and c == 0) else None,
                )
                pending[0] = (c, h, cps)
                if h < HPC - 1:
                    emit_qk_group(h + 1, c)
        if pending_ctx[0] is not None:
            emit_ctx(*pending_ctx[0])
            pending_ctx[0] = None
        while pending[0] is not None or pending_E:
            flush_pending(2)


def build_program(debug=False):
    nc = bacc.Bacc("TRN2", target_bir_lowering=False, debug=False)
    xT = nc.dram_tensor("xT", [C, N], bf16, kind="ExternalInput").ap()
    w_qk = nc.dram_tensor("w_qk", [C, 2 * HD * HPC], bf16, kind="ExternalInput").ap()
    w_v = nc.dram_tensor("w_v", [C, VW], bf16, kind="ExternalInput").ap()
    b_qk = nc.dram_tensor("b_qk", [P, HPC], f32, kind="ExternalInput").ap()
    w_p = nc.dram_tensor("w_p", [P, 2 * C], bf16, kind="ExternalInput").ap()
    vones = nc.dram_tensor("vones", [P, VW], bf16, kind="ExternalInput").ap()
    onesrow = nc.dram_tensor("onesrow", [P, P], bf16, kind="ExternalInput").ap()
    y = nc.dram_tensor("y", [N, C], bf16, kind="ExternalOutput").ap()
    with tile.TileContext(nc) as tc:
        _emit(tc, nc, xT, w_qk, w_v, b_qk, w_p, vones, onesrow, y)
    nc.compile()
    return nc


_CACHE = {}


def _get_program():
    if "nc" not in _CACHE:
        _CACHE["nc"] = build_program()
    return _CACHE["nc"]


def make_in_maps(x, W_qkv, b_qkv, W_proj):
    """Per-core input dicts implementing the (batch, head-group) sharding."""
    x = np.ascontiguousarray(np.asarray(x, np.float32))
    W_qkv = np.asarray(W_qkv, np.float32)
    b_qkv = np.asarray(b_qkv, np.float32)
    W_proj = np.asarray(W_proj, np.float32)
    scale = float(HD) ** -0.5

    Wq = W_qkv[0:C].reshape(H, HD, C)
    Wk = W_qkv[C : 2 * C].reshape(H, HD, C)
    Wv = W_qkv[2 * C : 3 * C].reshape(H, HD, C)
    bq = b_qkv[0:C].reshape(H, HD)
    bk = b_qkv[C : 2 * C].reshape(H, HD)

    vones_mask = np.zeros((P, VW), np.float32)
    for i in range(HPC):
        vones_mask[:, i * 65 + HD] = 1.0
    onesrow_arr = np.zeros((P, P), np.float32)
    onesrow_arr[HD, 0:HD] = 1.0

    in_maps = []
    for core in range(NCORES):
        b = core // CORES_PER_B
        hg = core % CORES_PER_B
        heads = list(range(hg * HPC, (hg + 1) * HPC))

        xT = np.ascontiguousarray(x[b].T).astype(ml_dtypes.bfloat16)  # [C, N]
        w_qk = np.empty((C, 2 * HD * HPC), np.float32)  # cast to bf16 below
        b_qk_arr = np.empty((HPC, P), np.float32)  # transposed to [P, HPC] below
        w_v = np.zeros((C, VW), np.float32)
        # packed projection weights [128, 2C]: cols 0:C = heads 0+1 stacked
        # (128 contraction rows); cols C:2C = head 2 in rows 0:64, zero pad
        w_p = np.zeros((P, 2 * C), np.float32)
        for i, h in enumerate(heads):
            w_qk[:, i * P : i * P + HD] = Wq[h].T * scale
            w_qk[:, i * P + HD : (i + 1) * P] = Wk[h].T
            b_qk_arr[i, 0:HD] = bq[h] * scale
            b_qk_arr[i, HD:P] = bk[h]
            w_v[:, i * 65 : i * 65 + HD] = Wv[h].T
            wp_h = W_proj[:, h * HD : (h + 1) * HD].T  # [HD, C]
            if i < 2:
                w_p[i * HD : (i + 1) * HD, 0:C] = wp_h
            else:
                w_p[0:HD, C : 2 * C] = wp_h
        in_maps.append(
            {"xT": xT,
             "w_qk": w_qk.astype(ml_dtypes.bfloat16),
             "w_v": w_v.astype(ml_dtypes.bfloat16),
             "b_qk": np.ascontiguousarray(b_qk_arr.T),
             "w_p": w_p.astype(ml_dtypes.bfloat16),
             "vones": vones_mask.astype(ml_dtypes.bfloat16),
             "onesrow": onesrow_arr.astype(ml_dtypes.bfloat16)}
        )
    return in_maps


def gather_output(results, b_qkv, W_proj, b_proj):
    """Sum the per-core partial projections (TP all-reduce) + effective bias."""
    out = np.zeros((B, N, C), np.float32)
    for core in range(NCORES):
        out[core // CORES_PER_B] += np.asarray(results[core]["y"], np.float32)
    b_v = np.asarray(b_qkv, np.float32)[2 * C : 3 * C]
    b_eff = np.asarray(b_proj, np.float32) + np.asarray(W_proj, np.float32) @ b_v
    out += b_eff
    return out


def kernel(x=None, xpos=None, W_qkv=None, b_qkv=None, W_proj=None, b_proj=None, **kw):
    del xpos, kw  # rope disabled in this configuration; xpos unused
    nc = _get_program()
    in_maps = make_in_maps(x, W_qkv, b_qkv, W_proj)
    res = run_bass_kernel_spmd(nc, in_maps, core_ids=list(range(NCORES)))
    return gather_output(res.results, b_qkv, W_proj, b_proj)



# revision 16
# speedup vs baseline: 1.0215x; 1.0215x over previous
"""Trainium2 Bass kernel for a 12-head attention block (B=2, N=2048, C=768).

Sharding: the 24 (batch, head) pairs are split across 8 NeuronCores —
4 cores per batch element, 3 heads per core (data + head/tensor parallel).
Each core computes qkv projections for its heads, the full attention for
its heads (the N x N score matrix is private to a core), and a *partial*
output projection over its heads' channels.  The host sums the 4 partial
projections per batch element (the tensor-parallel all-reduce) and adds
the bias.

Device algorithm (activations/weights bf16 — the PE streams its moving
operand at 1 column/cycle for 2-byte dtypes vs 2 cycles for fp32/fp32r —
with fp32 PSUM accumulation everywhere):

  xT [768, 2048] (x transposed on host)
  B:  qk^T  = W_qk^T.T @ xT  -> per-head tile [q^T(64 rows); k^T(64)] x 2048
      (attention scale 1/8 and b_q, b_k folded into W/bias on host)
  B2: v     = xT.T @ W_v^T   -> [2048, 3*65] with a column of ones per head
  C:  S^T[key, q] = k^T.T @ q^T        (per 128-key tile, 512-q chunk)
      P^T = exp(S^T)                   (ScalarE, no max subtraction:
                                        logits are in [-3, 3] by construction)
      ctx_u^T[d|den, q] += [v | 1].T @ P^T   (fused denominator row)
  D:  ctx^T = ctx_u^T[0:64] * (1/den)  (den broadcast across partitions via a
      ones-row matmul at base partition 64, then reciprocal_approx_fast)
  E:  y[n, :] += ctx^T.T @ W_p^T      (partial projection, summed on host)

Scheduling notes (why the emission order looks the way it does): the PE's
HAM clock gate re-throttles 2.4->1.2 GHz after ~3.4us of idle, so the
normalize/projection work for a tile is deferred and emitted mid-way
through the NEXT attention block, where the exp pipeline has buffered
work; a warm-up matmul spin bridges the input-DMA window; the exp table
set is preloaded with a dummy activation; S^T matmul pairs are emitted
back-to-back into opposite PE row groups so they overlap.
"""

import numpy as np
import ml_dtypes

import concourse.bacc as bacc
import concourse.tile as tile
import concourse.mybir as mybir
from concourse.bass_utils import run_bass_kernel_spmd

# Problem shape (hardcoded; harness contract)
B, N, C = 2, 2048, 768
H, HD = 12, 64
NCORES = 8
CORES_PER_B = NCORES // B      # 4
HPC = H // CORES_PER_B         # 3 heads per core
P = 128
NT = N // P                    # 16 key/n tiles
KT = C // P                    # 6 c_in tiles
CH = 512                       # q chunk (max fp32 moving free dim)
QCH = N // CH                  # 4 chunks
VW = 3 * 65                    # v width: 3 heads x (64 + fused ones column)

f32 = mybir.dt.float32
f32r = mybir.dt.float32r
bf16 = mybir.dt.bfloat16
EXP = mybir.ActivationFunctionType.Exp

def _emit(tc, nc, xT, w_qk, w_v, b_qk, w_p, vones, onesrow, y, dbg=None):
    from contextlib import ExitStack

    with ExitStack() as ctx:
        consts = ctx.enter_context(tc.tile_pool(name="consts", bufs=1))
        qk_pool = ctx.enter_context(tc.tile_pool(name="qk", bufs=HPC))
        qk2_pool = ctx.enter_context(tc.tile_pool(name="qk2", bufs=HPC))
        v_pool = ctx.enter_context(tc.tile_pool(name="v", bufs=NT))
        ctx_pool = ctx.enter_context(tc.tile_pool(name="ctxp", bufs=HPC))
        y_pool = ctx.enter_context(tc.tile_pool(name="y", bufs=3))
        ps_a = ctx.enter_context(tc.tile_pool(name="ps_a", bufs=2, space="PSUM"))
        ps_s = ctx.enter_context(tc.tile_pool(name="ps_s", bufs=2, space="PSUM"))
        ps_c = ctx.enter_context(tc.tile_pool(name="ps_c", bufs=2, space="PSUM"))

        # ---- memset-only tiles first (no DMA deps): the PE warm-up spin and
        # the exp table preload key off these, so both start right at the
        # preamble boundary instead of waiting for any input DMA.
        warm_sb = consts.tile([P, 2 * P], bf16)
        nc.vector.memset(warm_sb[:], 0.0)
        # persistent denominator-row tiles, zeroed once: the broadcast matmul
        # reads all 64 partitions (64:128) so the unused rows must be 0, not
        # SBUF garbage (NaN x 0 = NaN)
        denr_tiles = [
            consts.tile([P, CH], bf16, name=f"denr{_}") for _ in range(2)
        ]
        for t in denr_tiles:
            nc.vector.memset(t[:], 0.0)

        # preload the exp spline table set (~2.7us): FIRST instruction on the
        # Activation engine, before that engine's input DMAs
        actwarm = consts.tile([P, P], bf16)
        nc.scalar.activation(actwarm[:], warm_sb[:, 0:P], EXP)

        # SWDGE warm-up: the first gpsimd dma_start pays a ~6us Q7 IRAM load;
        # issue a tiny dummy SBUF->SBUF copy now so the mid-kernel qk2 swap
        # DMAs run at full speed.
        swdge_warm = consts.tile([1, 64], bf16)
        nc.gpsimd.dma_start(swdge_warm[:], warm_sb[0:1, 0:64])

        # ---- inputs.  The per-core HBM wire (~360 GB/s) is shared by every
        # queue, so the ~4.3MB of input takes ~12us on the wire regardless of
        # queue count; what matters is wire ORDER.  All large tensors go on
        # the sync queue in need-time order (wqk + x first halves gate the
        # first attention block); the scalar queue carries only the small
        # constants (~0.7MB) and then stays idle so the ACT engine's queue is
        # clear before the exp stream starts.  [128,1024] half-chunks keep
        # the 2KB-per-partition-line full DMA rate (1KB lines run at half).
        wqk_sb = consts.tile([P, KT, 2 * HD * HPC], bf16)
        nc.sync.dma_start(wqk_sb[:], w_qk.rearrange("(t p) m -> p t m", p=P))

        x_pool = ctx.enter_context(tc.tile_pool(name="x", bufs=KT))
        x_sb = [x_pool.tile([P, N], bf16, tag="x", name=f"x{_}") for _ in range(KT)]
        half = N // 2
        for kt in range(KT):
            nc.sync.dma_start(x_sb[kt][:, 0:half], xT[kt * P : (kt + 1) * P, 0:half])
        for kt in range(KT):
            nc.sync.dma_start(
                x_sb[kt][:, half:N], xT[kt * P : (kt + 1) * P, half:N]
            )
        vones_sb = consts.tile([P, VW], bf16)
        nc.sync.dma_start(vones_sb[:], vones[:])

        # scalar (Activation) queue: small constants only.  b_qk is
        # host-transposed to [128, 3] so the transfer is one clean descriptor
        # per partition (a strided 4-byte gather here poisons the queue).
        bqk_sb = consts.tile([P, HPC], f32)
        nc.scalar.dma_start(bqk_sb[:], b_qk[:])
        wv_sb = consts.tile([P, KT, VW], bf16)
        nc.scalar.dma_start(wv_sb[:], w_v.rearrange("(t p) m -> p t m", p=P))
        # projection weights, host-packed [128, 2C]: cols 0:C = heads 0+1
        # stacked; cols C:2C = head 2 zero-padded to 128 contraction rows so
        # every E matmul runs in (128,128) PE mode
        wp_sb = consts.tile([P, 2 * C], bf16)
        nc.scalar.dma_start(wp_sb[:], w_p[:])
        # [128,128] broadcast stationary: row 64 = [ones(64) | zeros(64)],
        # everything else zero.  Shaped so the den-broadcast matmul runs in
        # the same (64,128) PE tile mode as the S^T pairs / projection units
        # (row-tiling mode switches drain the PE array).
        onesrow_sb = consts.tile([P, P], bf16)
        nc.scalar.dma_start(onesrow_sb[:], onesrow[:])

        # persistent activations
        qk_sb = [qk_pool.tile([P, N], bf16, tag="qk", name=f"qk{_}") for _ in range(HPC)]
        qk2_sb = [qk2_pool.tile([P, N], bf16, tag="qk2", name=f"qk2_{_}") for _ in range(HPC)]
        v_sb = [v_pool.tile([P, VW], bf16, tag="v", name=f"v{_}") for _ in range(NT)]
        ctx01_sb = ctx_pool.tile([P, N], bf16, tag="ctx01", name="ctx01")
        ctx2_sb = ctx_pool.tile([P, N], bf16, tag="ctx2", name="ctx2")
        stage_pool = ctx.enter_context(tc.tile_pool(name="stage", bufs=2))

        nc.vector.memset(ctx2_sb[HD:P, :], 0.0)

        # PE warm-up: the HAM clock gate needs ~3.4us of sustained matmul
        # activity to lift the PE from 1.2 to 2.4 GHz, and re-throttles after
        # a ~3.4us idle window.  Spin dummy matmuls on the memset tile (no DMA
        # dep) so the spin starts at the preamble boundary; sized to end right
        # as the first x chunks land off the parallel DMA queues.
        wps = ps_a.tile([P, CH], f32, tag="ps_a", name="warm_ps")
        for _ in range(22):
            nc.tensor.matmul(
                wps[:, 0 : 2 * P], warm_sb[:, 0:P], warm_sb[:], start=True, stop=True
            )

        def emit_qk_group(t, cc):
            # qk^T head tile t, q-chunk cc: [q^T(64); k^T(64)] x CH
            sl = slice(cc * CH, (cc + 1) * CH)
            ps = ps_a.tile([P, CH], f32, tag="ps_a", name="ps_qk")
            for kt in range(KT):
                nc.tensor.matmul(
                    ps[:],
                    wqk_sb[:, kt, t * P : (t + 1) * P],
                    x_sb[kt][:, sl],
                    start=(kt == 0),
                    stop=(kt == KT - 1),
                )
            nc.vector.tensor_scalar_add(qk_sb[t][:, sl], ps[:], bqk_sb[:, t : t + 1])
            # swapped copy per chunk (k^T to partitions 0:64, q^T to 64:128);
            # on the GPSIMD SWDGE queue: SBUF->SBUF, so it costs no HBM wire
            # time and never queues behind the x/weight input stream
            nc.gpsimd.dma_start(qk2_sb[t][0:HD, sl], qk_sb[t][HD:P, sl])
            nc.gpsimd.dma_start(qk2_sb[t][HD:P, sl], qk_sb[t][0:HD, sl])

        def emit_v(nt):
            # v natural layout [key, 3*65] (+ ones columns)
            ps = ps_a.tile([P, CH], f32, tag="ps_a", name="ps_v")
            for kt in range(KT):
                nc.tensor.matmul(
                    ps[:, 0:VW],
                    x_sb[kt][:, nt * P : (nt + 1) * P],
                    wv_sb[:, kt, :],
                    start=(kt == 0),
                    stop=(kt == KT - 1),
                )
            nc.vector.tensor_add(v_sb[nt][:], ps[:, 0:VW], vones_sb[:])

        bc_pool = ctx.enter_context(tc.tile_pool(name="bc", bufs=2))
        p_pool = ctx.enter_context(tc.tile_pool(name="p", bufs=8))

        # ---- C/D/E: attention, normalize (deferred one step), projection.
        # S^T pairs go to opposite PE row groups (partitions 0:64 / 64:128)
        # and run concurrently; exp processes both halves in one ACTIVATE.
        # The normalize (D) for tile (c,h) is emitted only after the next
        # C-block so its reciprocal never stalls the PE (a >3.4us PE gap
        # re-throttles the HAM clock gate to 1.2 GHz).
        # deferred-ctx slot: (cps, h, kp, pt) of the S^T pair whose P@V
        # matmuls have not been emitted yet.  Deferring the ctx of pair j
        # until after pair j+1's S^T matmuls gives EXP(j) a full iteration
        # of slack, so the first ctx matmul never stalls on the exp sem.
        pending_ctx = [None]

        def emit_ctx(cps, h, kp, pt):
            kt0, kt1 = 2 * kp, 2 * kp + 1
            nc.tensor.matmul(
                cps[:],
                v_sb[kt0][:, h * 65 : (h + 1) * 65],
                pt[:, 0:CH],
                start=(kp == 0),
                stop=False,
            )
            nc.tensor.matmul(
                cps[:],
                v_sb[kt1][:, h * 65 : (h + 1) * 65],
                pt[:, CH : 2 * CH],
                start=False,
                stop=(kp == NT // 2 - 1),
            )

        def emit_C(c, h, first=False, flush=None, qk_feed=None):
            cps = ps_c.tile([65, CH], f32, tag="ps_c", name="cps")
            for kp in range(NT // 2):
                kt0, kt1 = 2 * kp, 2 * kp + 1
                if qk_feed is not None and kp in (1, 3, 5):
                    # first block: the remaining q-chunk projections are
                    # emitted just ahead of the S^T pairs that consume their
                    # key columns, so the block starts after chunk 0 lands
                    qk_feed(kp // 2 + 1)
                if first and kp >= 1:
                    # just-in-time v projection (v0/v1 are emitted before the
                    # C loop while the q/k swap DMA is still in flight)
                    emit_v(kt0)
                    emit_v(kt1)
                sps = ps_s.tile([P, 2 * CH], f32, tag="ps_s", name="sps")
                nc.tensor.matmul(
                    sps[:, 0:CH],
                    qk2_sb[h][0:HD, kt0 * P : (kt0 + 1) * P],
                    qk_sb[h][0:HD, c * CH : (c + 1) * CH],
                )
                nc.tensor.matmul(
                    sps[:, CH : 2 * CH],
                    qk_sb[h][HD:P, kt1 * P : (kt1 + 1) * P],
                    qk2_sb[h][HD:P, c * CH : (c + 1) * CH],
                )
                pt = p_pool.tile([P, 2 * CH], bf16, tag="p", name="pt")
                nc.scalar.activation(pt[:], sps[:], EXP)
                if pending_ctx[0] is not None:
                    emit_ctx(*pending_ctx[0])
                pending_ctx[0] = (cps, h, kp, pt)
                if kp % 2 == 1 and flush is not None:
                    # emit the previous tile's normalize / projection units
                    # here, in the (128,128) PE mode window after the ctx
                    # matmuls (mode switches drain the array).  Not at kp 0:
                    # ACT has no buffered work at a block start, so PE detours
                    # there starve the exp stream.  The last head's blocks
                    # drain the E backlog at double rate so the post-EXP tail
                    # only carries the final chunk's units.
                    flush(2 if h == HPC - 1 else 1)
            return cps

        dcount = [0]

        def emit_D(c, h, cps):
            denr = denr_tiles[dcount[0] % 2]
            dcount[0] += 1
            nc.vector.tensor_copy(denr[64:65, :], cps[64:65, :])
            bps = ps_a.tile([P, CH], f32, tag="ps_a", name="bps")
            # contraction padded to 128 rows (onesrow/denr rows other than 64
            # are zero) so the broadcast runs in (128,128) PE mode like the
            # ctx / projection matmuls — no row-tiling mode switch
            nc.tensor.matmul(
                bps[:], onesrow_sb[:], denr[:],
                start=True, stop=True,
            )
            bc = bc_pool.tile([HD, CH], f32, tag="bc", name="bc")
            # reciprocal straight from PSUM (custom DVE ops accept PSUM
            # sources) — drops the staging copy from the D critical chain
            nc.vector.reciprocal_approx_fast(bc[:], bps[0:HD, :])
            sl = slice(c * CH, (c + 1) * CH)
            if h == 0:
                nc.vector.tensor_mul(ctx01_sb[0:HD, sl], cps[0:HD, :], bc[:])
            elif h == 1:
                # stage + DMA into partitions 64:128 of the stacked tile
                # (DVE lanes are partition-locked; only DMA crosses halves)
                stg = stage_pool.tile([HD, CH], bf16, tag="stg", name="stg")
                nc.vector.tensor_mul(stg[:], cps[0:HD, :], bc[:])
                nc.gpsimd.dma_start(ctx01_sb[HD:P, sl], stg[:])
            else:
                nc.vector.tensor_mul(ctx2_sb[0:HD, sl], cps[0:HD, :], bc[:])

        def emit_E_unit(nt, tail=False):
            # partial projection for one n-tile: heads 0+1 contract 128 deep
            # via the stacked ctx01 tile; head 2 is zero-padded to 128 rows.
            # Mid-kernel units use the ps_a ring and evacuate on DVE (keeps
            # the bottleneck ACT queue free for exps).  Tail units (after the
            # last exp) instead use the now-idle ps_s ring -- one [P, 2CH]
            # buffer holds both halves, so unit i+1's matmuls never wait on
            # unit i's evacuation -- and split the evacuation DVE/ScalarE.
            nsl = slice(nt * P, (nt + 1) * P)
            if tail:
                ts = ps_s.tile([P, 2 * CH], f32, tag="ps_s", name="sps")
                psA, psB = ts[:, 0:CH], ts[:, CH : 2 * CH]
            else:
                psA = ps_a.tile([P, CH], f32, tag="ps_a", name="psA")[:]
                psB = ps_a.tile([P, CH], f32, tag="ps_a", name="psB")[:]
            nc.tensor.matmul(psA, ctx01_sb[:, nsl], wp_sb[:, 0:CH], start=True, stop=False)
            nc.tensor.matmul(psA, ctx2_sb[:, nsl], wp_sb[:, C : C + CH], start=False, stop=True)
            nc.tensor.matmul(psB[:, 0 : C - CH], ctx01_sb[:, nsl], wp_sb[:, CH:C], start=True, stop=False)
            nc.tensor.matmul(psB[:, 0 : C - CH], ctx2_sb[:, nsl], wp_sb[:, C + CH : 2 * C], start=False, stop=True)
            ysb = y_pool.tile([P, C], bf16, tag="y", name="ysb")
            nc.vector.tensor_copy(ysb[:, 0:CH], psA)
            if tail:
                nc.scalar.copy(ysb[:, CH:C], psB[:, 0 : C - CH])
            else:
                nc.vector.tensor_copy(ysb[:, CH:C], psB[:, 0 : C - CH])
            nc.sync.dma_start(y[nt * P : (nt + 1) * P, :], ysb[:])

        pending = [None]  # (c, h, cps) awaiting D
        pending_E = []  # (c, nt) projection units ready to emit

        def flush_pending(budget=1, tail=False):
            if pending[0] is not None:
                pc, ph, pcps = pending[0]
                emit_D(pc, ph, pcps)
                if ph == HPC - 1:
                    pending_E.extend(
                        pc * (CH // P) + i for i in range(CH // P)
                    )
                pending[0] = None
                budget -= 1
            while budget > 0 and pending_E:
                emit_E_unit(pending_E.pop(0), tail=tail)
                budget -= 1

        # Head-outer schedule: all 4 q-chunks of head h, then head h+1.
        # The next head's qk projection is emitted one chunk-group at a time
        # underneath the current (ACT-bound) attention blocks.
        emit_qk_group(0, 0)
        emit_v(0)
        emit_v(1)
        for h in range(HPC):
            for c in range(QCH):
                cps = emit_C(
                    c, h, first=(h == 0 and c == 0), flush=flush_pending,
                    qk_feed=(lambda cc: emit_qk_group(0, cc)) if (h == 0 and c == 0) else None,
                )
                pending[0] = (c, h, cps)
                if h < HPC - 1:
                    emit_qk_group(h + 1, c)
        if pending_ctx[0] is not None:
            emit_ctx(*pending_ctx[0])
            pending_ctx[0] = None
        while pending[0] is not None or pending_E:
            flush_pending(2, tail=True)


def build_program(debug=False):
    nc = bacc.Bacc("TRN2", target_bir_lowering=False, debug=False)
    xT = nc.dram_tensor("xT", [C, N], bf16, kind="ExternalInput").ap()
    w_qk = nc.dram_tensor("w_qk", [C, 2 * HD * HPC], bf16, kind="ExternalInput").ap()
    w_v = nc.dram_tensor("w_v", [C, VW], bf16, kind="ExternalInput").ap()
    b_qk = nc.dram_tensor("b_qk", [P, HPC], f32, kind="ExternalInput").ap()
    w_p = nc.dram_tensor("w_p", [P, 2 * C], bf16, kind="ExternalInput").ap()
    vones = nc.dram_tensor("vones", [P, VW], bf16, kind="ExternalInput").ap()
    onesrow = nc.dram_tensor("onesrow", [P, P], bf16, kind="ExternalInput").ap()
    y = nc.dram_tensor("y", [N, C], bf16, kind="ExternalOutput").ap()
    with tile.TileContext(nc) as tc:
        _emit(tc, nc, xT, w_qk, w_v, b_qk, w_p, vones, onesrow, y)
    nc.compile()
    return nc


_CACHE = {}


def _get_program():
    if "nc" not in _CACHE:
        _CACHE["nc"] = build_program()
    return _CACHE["nc"]


def make_in_maps(x, W_qkv, b_qkv, W_proj):
    """Per-core input dicts implementing the (batch, head-group) sharding."""
    x = np.ascontiguousarray(np.asarray(x, np.float32))
    W_qkv = np.asarray(W_qkv, np.float32)
    b_qkv = np.asarray(b_qkv, np.float32)
    W_proj = np.asarray(W_proj, np.float32)
    scale = float(HD) ** -0.5

    Wq = W_qkv[0:C].reshape(H, HD, C)
    Wk = W_qkv[C : 2 * C].reshape(H, HD, C)
    Wv = W_qkv[2 * C : 3 * C].reshape(H, HD, C)
    bq = b_qkv[0:C].reshape(H, HD)
    bk = b_qkv[C : 2 * C].reshape(H, HD)

    vones_mask = np.zeros((P, VW), np.float32)
    for i in range(HPC):
        vones_mask[:, i * 65 + HD] = 1.0
    onesrow_arr = np.zeros((P, P), np.float32)
    onesrow_arr[HD, 0:HD] = 1.0

    in_maps = []
    for core in range(NCORES):
        b = core // CORES_PER_B
        hg = core % CORES_PER_B
        heads = list(range(hg * HPC, (hg + 1) * HPC))

        xT = np.ascontiguousarray(x[b].T).astype(ml_dtypes.bfloat16)  # [C, N]
        w_qk = np.empty((C, 2 * HD * HPC), np.float32)  # cast to bf16 below
        b_qk_arr = np.empty((HPC, P), np.float32)  # transposed to [P, HPC] below
        w_v = np.zeros((C, VW), np.float32)
        # packed projection weights [128, 2C]: cols 0:C = heads 0+1 stacked
        # (128 contraction rows); cols C:2C = head 2 in rows 0:64, zero pad
        w_p = np.zeros((P, 2 * C), np.float32)
        for i, h in enumerate(heads):
            w_qk[:, i * P : i * P + HD] = Wq[h].T * scale
            w_qk[:, i * P + HD : (i + 1) * P] = Wk[h].T
            b_qk_arr[i, 0:HD] = bq[h] * scale
            b_qk_arr[i, HD:P] = bk[h]
            w_v[:, i * 65 : i * 65 + HD] = Wv[h].T
            wp_h = W_proj[:, h * HD : (h + 1) * HD].T  # [HD, C]
            if i < 2:
                w_p[i * HD : (i + 1) * HD, 0:C] = wp_h
            else:
                w_p[0:HD, C : 2 * C] = wp_h
        in_maps.append(
            {"xT": xT,
             "w_qk": w_qk.astype(ml_dtypes.bfloat16),
             "w_v": w_v.astype(ml_dtypes.bfloat16),
             "b_qk": np.ascontiguousarray(b_qk_arr.T),
             "w_p": w_p.astype(ml_dtypes.bfloat16),
             "vones": vones_mask.astype(ml_dtypes.bfloat16),
             "onesrow": onesrow_arr.astype(ml_dtypes.bfloat16)}
        )
    return in_maps


def gather_output(results, b_qkv, W_proj, b_proj):
    """Sum the per-core partial projections (TP all-reduce) + effective bias."""
    out = np.zeros((B, N, C), np.float32)
    for core in range(NCORES):
        out[core // CORES_PER_B] += np.asarray(results[core]["y"], np.float32)
    b_v = np.asarray(b_qkv, np.float32)[2 * C : 3 * C]
    b_eff = np.asarray(b_proj, np.float32) + np.asarray(W_proj, np.float32) @ b_v
    out += b_eff
    return out


def kernel(x=None, xpos=None, W_qkv=None, b_qkv=None, W_proj=None, b_proj=None, **kw):
    del xpos, kw  # rope disabled in this configuration; xpos unused
    nc = _get_program()
    in_maps = make_in_maps(x, W_qkv, b_qkv, W_proj)
    res = run_bass_kernel_spmd(nc, in_maps, core_ids=list(range(NCORES)))
    return gather_output(res.results, b_qkv, W_proj, b_proj)

